# revision 1
# baseline (speedup 1.0000x reference)
import numpy as np
import concourse.bass as bass
import concourse.mybir as mybir
import concourse.tile as tile
from concourse import bacc
from concourse.bass_utils import run_bass_kernel_spmd

B, S, D, H, DH = 4, 2048, 768, 12, 64
HPC = 6          # heads per core
PAIRS = HPC // 2
THETA = 10000.0
N_CORES = 8
F32 = mybir.dt.float32
F32R = mybir.dt.float32r
VW = HPC * 65    # V tile width per t-block: 6 heads x (64 + ones col)

_NC = None


def build_nc(with_collective=True):
    nc = bacc.Bacc("TRN2", target_bir_lowering=False, debug=False,
                   num_devices=N_CORES)
    xT = nc.dram_tensor("xT", [D, S], F32R, kind="ExternalInput")
    wqT = nc.dram_tensor("wqT", [D, 384], F32R, kind="ExternalInput")
    wkT = nc.dram_tensor("wkT", [D, 384], F32R, kind="ExternalInput")
    wv = nc.dram_tensor("wv", [D, VW], F32R, kind="ExternalInput")
    wo = nc.dram_tensor("wo", [384, D], F32R, kind="ExternalInput")
    cosd = nc.dram_tensor("cos", [128, S], F32, kind="ExternalInput")
    sind = nc.dram_tensor("sin", [128, S], F32, kind="ExternalInput")
    maskd = nc.dram_tensor("mask", [128, 4 * 512], F32, kind="ExternalInput")
    onesd = nc.dram_tensor("ones", [1, 64], F32R, kind="ExternalInput")
    out = nc.dram_tensor("out", [S, D], F32, kind="ExternalOutput")

    with tile.TileContext(nc) as tc:
        with tc.tile_pool(name="persist", bufs=1) as pp, \
             tc.tile_pool(name="dram", bufs=1, space="DRAM") as dpool:
            sb_q = [pp.tile([128, S], F32R, name=f"sb_q{i}") for i in range(PAIRS)]
            sb_k = [pp.tile([128, S], F32R, name=f"sb_k{i}") for i in range(PAIRS)]
            sb_v = pp.tile([128, 16 * VW], F32R)
            sb_mask = pp.tile([128, 128], F32)
            sb_ones = pp.tile([1, 64], F32R)
            ones128 = pp.tile([128, 1], F32)
            nc.vector.memset(ones128[:], 1.0)
            bounce_in = dpool.tile([S, D], F32)
            bounce_out = dpool.tile([S, D], F32)

            nc.sync.dma_start(sb_mask[:], maskd[:, 0:128])
            nc.sync.dma_start(sb_ones[:], onesd[:])

            # ---- phase A/B: projections + RoPE, streaming x by col-block ----
            with tc.tile_pool(name="stage", bufs=1) as sp, \
                 tc.tile_pool(name="xp", bufs=2) as xp, \
                 tc.tile_pool(name="pqc", bufs=3, space="PSUM") as pqc, \
                 tc.tile_pool(name="pv", bufs=2, space="PSUM") as pv, \
                 tc.tile_pool(name="rtmp", bufs=1) as rtmp:
                sb_wq = sp.tile([128, 6 * 384], F32R)
                sb_wk = sp.tile([128, 6 * 384], F32R)
                sb_wv = sp.tile([128, 6 * VW], F32R)
                sb_cos = sp.tile([128, S], F32)
                sb_sin = sp.tile([128, S], F32)
                uh = sp.tile([128, 2 * PAIRS * 1024], F32)

                for ck in range(6):
                    for sb_w, wd in ((sb_wq, wqT), (sb_wk, wkT)):
                        nc.sync.dma_start(sb_w[:, ck * 384:(ck + 1) * 384],
                                          wd[ck * 128:(ck + 1) * 128, :])
                    nc.sync.dma_start(sb_wv[:, ck * VW:(ck + 1) * VW],
                                      wv[ck * 128:(ck + 1) * 128, :])
                nc.sync.dma_start(sb_cos[:], cosd[:])
                nc.sync.dma_start(sb_sin[:], sind[:])

                for tt in range(4):
                    xcol = xp.tile([128, 6 * 512], F32R)
                    for ck in range(6):
                        nc.sync.dma_start(
                            xcol[:, ck * 512:(ck + 1) * 512],
                            xT[ck * 128:(ck + 1) * 128,
                               tt * 512:(tt + 1) * 512])
                    csl = sb_cos[:, tt * 512:(tt + 1) * 512]
                    ssl = sb_sin[:, tt * 512:(tt + 1) * 512]
                    half = tt % 2
                    for wi, (wc, dst) in enumerate(((sb_wq, sb_q),
                                                    (sb_wk, sb_k))):
                        for p in range(PAIRS):
                            pc = pqc.tile([128, 512], F32)
                            for ck in range(6):
                                xs = xcol[:, ck * 512:(ck + 1) * 512]
                                nc.tensor.matmul(
                                    pc[:],
                                    wc[:, ck * 384 + p * 128:
                                          ck * 384 + (p + 1) * 128],
                                    xs, start=(ck == 0), stop=(ck == 5))
                            nc.vector.tensor_mul(
                                dst[p][:, tt * 512:(tt + 1) * 512],
                                pc[:], csl)
                            ub = wi * 3072 + p * 1024 + half * 512
                            nc.vector.tensor_mul(
                                uh[:, ub:ub + 512], pc[:], ssl)
                    for tj in range(4):
                        tb = tt * 4 + tj
                        pvt = pv.tile([128, VW], F32)
                        for ck in range(6):
                            nc.tensor.matmul(
                                pvt[:],
                                xcol[:, ck * 512 + tj * 128:
                                        ck * 512 + tj * 128 + 128],
                                sb_wv[:, ck * VW:(ck + 1) * VW],
                                start=(ck == 0), stop=(ck == 5))
                        with nc.allow_low_precision(reason="f32r V store"):
                            nc.vector.tensor_copy(
                                sb_v[:, tb * VW:(tb + 1) * VW], pvt[:])
                        for h in range(HPC):
                            col = tb * VW + h * 65 + 64
                            nc.scalar.copy(sb_v[:, col:col + 1], ones128[:])
                    if tt % 2 == 1:
                        base = (tt - 1) * 512
                        swf = rtmp.tile([128, 2 * PAIRS * 1024], F32)
                        for g in range(4):
                            nc.sync.dma_start(
                                swf[g * 32:(g + 1) * 32, :],
                                uh[(g ^ 1) * 32:((g ^ 1) + 1) * 32, :])
                        for wi, dsts in ((0, sb_q), (1, sb_k)):
                            for p in range(PAIRS):
                                sl = swf[:, wi * 3072 + p * 1024:
                                            wi * 3072 + (p + 1) * 1024]
                                nc.vector.tensor_add(
                                    dsts[p][:, base:base + 1024],
                                    dsts[p][:, base:base + 1024], sl)

            # ---- phases C/D: attention + output projection ----
            with tc.tile_pool(name="late", bufs=1) as lp:
                sb_ctx = [lp.tile([128, S], F32R, name=f"sb_ctx{i}") for i in range(3)]
                sb_wo = lp.tile([128, 3 * D], F32R)
                for ci in range(3):
                    nc.sync.dma_start(sb_wo[:, ci * D:(ci + 1) * D],
                                      wo[ci * 128:(ci + 1) * 128, :])

                with tc.tile_pool(name="pscore", bufs=3, space="PSUM") as pps, \
                     tc.tile_pool(name="pctx", bufs=2, space="PSUM") as ppc, \
                     tc.tile_pool(name="pbr", bufs=1, space="PSUM") as pbp, \
                     tc.tile_pool(name="po", bufs=2, space="PSUM") as po, \
                     tc.tile_pool(name="et", bufs=3) as ep, \
                     tc.tile_pool(name="ot", bufs=2) as ot, \
                     tc.tile_pool(name="nrm", bufs=2) as nrm:

                    def score_block(h, p, off, qt, kb):
                        # diag blocks (j>=0): cols below j*128 are fully
                        # masked -> skip them in matmul/exp; only the first
                        # surviving 128-col chunk needs the tril mask
                        j = kb - 4 * qt
                        lo = max(j, 0) * 128
                        psc = pps.tile([128, 512], F32)
                        nc.tensor.matmul(
                            psc[:, lo:],
                            sb_k[p][off:off + 64, kb * 128:(kb + 1) * 128],
                            sb_q[p][off:off + 64,
                                    qt * 512 + lo:(qt + 1) * 512],
                            start=True, stop=True)
                        et = ep.tile([128, 512], F32R)
                        nc.scalar.activation(et[:, lo:], psc[:, lo:],
                                             mybir.ActivationFunctionType.Exp)
                        if j >= 0:
                            nc.vector.tensor_mul(
                                et[:, lo:lo + 128], et[:, lo:lo + 128],
                                sb_mask[:, 0:128])
                        return et, lo

                    for qt in range(4):
                        for h in range(HPC):
                            p, off = h // 2, (h % 2) * 64
                            pctx = ppc.tile([65, 512], F32)
                            nkb = 4 * qt + 4
                            prev, plo = score_block(h, p, off, qt, 0)
                            for kb in range(1, nkb + 1):
                                if kb < nkb:
                                    nxt, nlo = score_block(h, p, off, qt, kb)
                                nc.tensor.matmul(
                                    pctx[:, plo:],
                                    sb_v[:, (kb - 1) * VW + h * 65:
                                            (kb - 1) * VW + h * 65 + 65],
                                    prev[:, plo:],
                                    start=(kb == 1), stop=(kb == nkb),
                                    skip_group_check=True)
                                if kb < nkb:
                                    prev, plo = nxt, nlo
                            rc = nrm.tile([1, 512], F32R)
                            with nc.allow_low_precision(
                                    reason="f32r feed to broadcast matmul"):
                                nc.vector.reciprocal(rc[:], pctx[64:65, :])
                            pbr = pbp.tile([64, 512], F32)
                            nc.tensor.matmul(pbr[:], sb_ones[:], rc[:],
                                             start=True, stop=True)
                            sc = nrm.tile([64, 512], F32)
                            nc.vector.tensor_copy(sc[:], pctx[0:64, :])
                            nc.vector.tensor_mul(
                                sb_ctx[p][off:off + 64,
                                          qt * 512:(qt + 1) * 512],
                                sc[:], pbr[:])
                        for tj in range(4):
                            tb = qt * 4 + tj
                            obuf = ot.tile([128, D], F32)
                            for nn2 in range(2):
                                pot = po.tile([128, 384], F32)
                                for ci in range(3):
                                    nc.tensor.matmul(
                                        pot[:],
                                        sb_ctx[ci][:, tb * 128:(tb + 1) * 128],
                                        sb_wo[:, ci * D + nn2 * 384:
                                                ci * D + nn2 * 384 + 384],
                                        start=(ci == 0), stop=(ci == 2))
                                nc.vector.tensor_copy(
                                    obuf[:, nn2 * 384:(nn2 + 1) * 384], pot[:])
                            nc.sync.dma_start(
                                bounce_in[tb * 128:(tb + 1) * 128, :],
                                obuf[:])

            if with_collective:
                nc.gpsimd.collective_compute(
                    "AllReduce", mybir.AluOpType.add,
                    replica_groups=[[0, 1], [2, 3], [4, 5], [6, 7]],
                    ins=[bounce_in.opt()], outs=[bounce_out.opt()])
                nc.sync.dma_start(out[:], bounce_out[:])
            else:
                nc.sync.dma_start(out[:], bounce_in[:])
    nc.compile()
    return nc


def make_in_maps(x, w_q, w_k, w_v, w_o, token_positions):
    even = np.arange(0, 64, 2)
    odd = np.arange(1, 64, 2)
    perm_eo = np.concatenate([even, odd])
    pos = np.asarray(token_positions).astype(np.float32)
    inv = THETA ** (-np.arange(32, dtype=np.float32) / 32.0)
    ang = inv[:, None] * pos[None, :]
    c32 = np.cos(ang).astype(np.float32)
    s32 = np.sin(ang).astype(np.float32)
    cosd = np.tile(c32, (4, 1))
    sind = np.concatenate([s32, -s32, s32, -s32], axis=0)
    kloc = np.arange(128)[:, None]
    qloc = np.arange(512)[None, :]
    maskd = np.concatenate(
        [(kloc + j * 128 <= qloc).astype(np.float32) for j in range(4)],
        axis=1)
    onesd = np.ones((1, 64), np.float32)
    xn = np.asarray(x, dtype=np.float32)
    wqn = np.asarray(w_q, dtype=np.float32)
    wkn = np.asarray(w_k, dtype=np.float32)
    wvn = np.asarray(w_v, dtype=np.float32)
    won = np.asarray(w_o, dtype=np.float32)
    in_maps = []
    for c in range(N_CORES):
        b, hg = c // 2, c % 2
        heads = hg * HPC + np.arange(HPC)
        rows_eo = (heads[:, None] * 64 + perm_eo[None, :]).reshape(-1)
        wv_r = np.zeros((D, VW), np.float32)
        for h in range(HPC):
            g = hg * HPC + h
            wv_r[:, h * 65:h * 65 + 64] = wvn[g * 64:(g + 1) * 64, :].T
        in_maps.append({
            "xT": np.ascontiguousarray(xn[b].T),
            "wqT": np.ascontiguousarray((wqn[rows_eo] * 0.125).T),
            "wkT": np.ascontiguousarray(wkn[rows_eo].T),
            "wv": wv_r,
            "wo": np.ascontiguousarray(won[:, hg * 384:(hg + 1) * 384].T),
            "cos": cosd,
            "sin": sind,
            "mask": maskd,
            "ones": onesd,
        })
    return in_maps


def kernel(x, w_q, w_k, w_v, w_o, token_positions):
    global _NC
    if _NC is None:
        _NC = build_nc()
    in_maps = make_in_maps(x, w_q, w_k, w_v, w_o, token_positions)
    res = run_bass_kernel_spmd(_NC, in_maps, core_ids=list(range(N_CORES)))
    return np.stack([res.results[2 * b]["out"] for b in range(B)], axis=0)



# revision 3
# speedup vs baseline: 1.1333x; 1.1333x over previous
import numpy as np
import ml_dtypes
import concourse.bass as bass
import concourse.mybir as mybir
import concourse.tile as tile
from concourse import bacc
from concourse.bass_utils import run_bass_kernel_spmd
from concourse.alu_op_type import AluOpType

B, S, D = 4, 2048, 768
HPC = 6            # heads per core
PAIRS = 3
THETA = 10000.0
N_CORES = 8
F32 = mybir.dt.float32
BF16 = mybir.dt.bfloat16
BF = ml_dtypes.bfloat16
VW = HPC * 65      # 390: per-tb V tile width (6 heads x (64 dims + ones col))
EXP = mybir.ActivationFunctionType.Exp

_NC = None


def build_nc(with_collective=True):
    nc = bacc.Bacc("TRN2", target_bir_lowering=False, debug=False,
                   num_devices=N_CORES)
    xd = nc.dram_tensor("xd", [128, 4 * 3072], BF16, kind="ExternalInput")
    wqd = nc.dram_tensor("wqd", [128, 2304], BF16, kind="ExternalInput")
    wkd = nc.dram_tensor("wkd", [128, 2304], BF16, kind="ExternalInput")
    wvd = nc.dram_tensor("wvd", [128, 6 * VW], BF16, kind="ExternalInput")
    wod = nc.dram_tensor("wod", [128, 2304], BF16, kind="ExternalInput")
    cosd = nc.dram_tensor("cos", [128, S], BF16, kind="ExternalInput")
    sind = nc.dram_tensor("sin", [128, S], BF16, kind="ExternalInput")
    maskd = nc.dram_tensor("mask", [128, 128], BF16, kind="ExternalInput")
    out = nc.dram_tensor("out", [S, D], F32, kind="ExternalOutput")

    with tile.TileContext(nc) as tc:
        with tc.tile_pool(name="persist", bufs=1) as pp, \
             tc.tile_pool(name="dram", bufs=1, space="DRAM") as dpool, \
             tc.tile_pool(name="uhp", bufs=2) as uhp, \
             tc.tile_pool(name="swp", bufs=2) as swp, \
             tc.tile_pool(name="etp", bufs=4) as etp, \
             tc.tile_pool(name="ctxtp", bufs=8) as ctxtp, \
             tc.tile_pool(name="rcpp", bufs=4) as rcpp, \
             tc.tile_pool(name="scrp", bufs=4) as scrp, \
             tc.tile_pool(name="stgp", bufs=3) as stgp, \
             tc.tile_pool(name="obp", bufs=2) as obp, \
             tc.tile_pool(name="pqc", bufs=2, space="PSUM") as pqc, \
             tc.tile_pool(name="pscore", bufs=2, space="PSUM") as pscore, \
             tc.tile_pool(name="pv", bufs=1, space="PSUM") as pvp, \
             tc.tile_pool(name="po", bufs=1, space="PSUM") as pop, \
             tc.tile_pool(name="pctx", bufs=1, space="PSUM") as pctxp:

            sb_q = [pp.tile([128, S], BF16, name=f"sb_q{i}") for i in range(PAIRS)]
            sb_k = [pp.tile([128, S], BF16, name=f"sb_k{i}") for i in range(PAIRS)]
            sb_v = pp.tile([128, 16 * VW], BF16)
            sb_ctx = [pp.tile([128, S], BF16, name=f"sb_ctx{i}") for i in range(PAIRS)]
            xcol = [pp.tile([128, 3072], BF16, name=f"xcol{i}") for i in range(4)]
            sb_wq = pp.tile([128, 2304], BF16)
            sb_wk = pp.tile([128, 2304], BF16)
            sb_wv = pp.tile([128, 6 * VW], BF16)
            sb_wo = pp.tile([128, 2304], BF16)
            sb_cos = pp.tile([128, S], BF16)
            sb_sin = pp.tile([128, S], BF16)
            sb_mask = pp.tile([128, 128], BF16)
            bounce_in = dpool.tile([S, D], F32)
            bounce_out = dpool.tile([S, D], F32)

            # input loads, most-urgent first
            nc.sync.dma_start(xcol[0][:], xd[:, 0:3072])
            nc.sync.dma_start(sb_wk[:], wkd[:])
            nc.sync.dma_start(sb_wq[:], wqd[:])
            nc.sync.dma_start(sb_cos[:], cosd[:])
            nc.sync.dma_start(sb_sin[:], sind[:])
            nc.sync.dma_start(sb_mask[:], maskd[:])
            nc.sync.dma_start(sb_wv[:], wvd[:])
            nc.sync.dma_start(xcol[1][:], xd[:, 3072:6144])
            nc.sync.dma_start(xcol[2][:], xd[:, 6144:9216])
            nc.sync.dma_start(xcol[3][:], xd[:, 9216:12288])
            nc.sync.dma_start(sb_wo[:], wod[:])

            def proj_tt(tt):
                """QKV projection + RoPE for token block tt (512 tokens)."""
                xc = xcol[tt]
                csl = sb_cos[:, tt * 512:(tt + 1) * 512]
                ssl = sb_sin[:, tt * 512:(tt + 1) * 512]
                uh = uhp.tile([128, 3072], BF16)
                for wi, wsb, dst in ((0, sb_wk, sb_k), (1, sb_wq, sb_q)):
                    for pr in range(PAIRS):
                        pc = pqc.tile([128, 512], F32)
                        for ck in range(6):
                            nc.tensor.matmul(
                                pc[:],
                                wsb[:, ck * 384 + pr * 128:
                                       ck * 384 + (pr + 1) * 128],
                                xc[:, ck * 512:(ck + 1) * 512],
                                start=(ck == 0), stop=(ck == 5))
                        with nc.allow_low_precision(reason="bf16 qk"):
                            nc.vector.tensor_mul(
                                dst[pr][:, tt * 512:(tt + 1) * 512],
                                pc[:], csl)
                            nc.vector.tensor_mul(
                                uh[:, (wi * 3 + pr) * 512:
                                      (wi * 3 + pr + 1) * 512],
                                pc[:], ssl)
                        yield
                # partition swap of the sin part (32-part groups g <-> g^1)
                swf = swp.tile([128, 3072], BF16)
                for g in range(4):
                    nc.sync.dma_start(swf[g * 32:(g + 1) * 32, :],
                                      uh[(g ^ 1) * 32:((g ^ 1) + 1) * 32, :])
                for wi, dst in ((0, sb_k), (1, sb_q)):
                    for pr in range(PAIRS):
                        sl = swf[:, (wi * 3 + pr) * 512:(wi * 3 + pr + 1) * 512]
                        d = dst[pr][:, tt * 512:(tt + 1) * 512]
                        with nc.allow_low_precision(reason="bf16 qk add"):
                            nc.vector.tensor_add(d, d, sl)
                    yield
                # V projection per 128-token block
                for tj in range(4):
                    tb = tt * 4 + tj
                    pvt = pvp.tile([128, VW], F32)
                    for ck in range(6):
                        nc.tensor.matmul(
                            pvt[:],
                            xc[:, ck * 512 + tj * 128:
                                  ck * 512 + tj * 128 + 128],
                            sb_wv[:, ck * VW:(ck + 1) * VW],
                            start=(ck == 0), stop=(ck == 5))
                    with nc.allow_low_precision(reason="bf16 v"):
                        nc.vector.tensor_copy(sb_v[:, tb * VW:(tb + 1) * VW],
                                              pvt[:])
                    nc.vector.memset(sb_v[:, tb * VW + 64:(tb + 1) * VW:65], 1.0)
                    yield

            def finish_tb(qt, qj, ctxt):
                """Transpose ctx to [hd, tok], output projection, store."""
                tb = qt * 4 + qj
                for c in range(PAIRS):
                    nc.vector.transpose(
                        sb_ctx[c][:, tb * 128:(tb + 1) * 128],
                        ctxt[:, c * 128:(c + 1) * 128])
                ob = obp.tile([128, D], F32)
                for nn in range(2):
                    pot = pop.tile([128, 384], F32)
                    for ci in range(PAIRS):
                        nc.tensor.matmul(
                            pot[:],
                            sb_ctx[ci][:, tb * 128:(tb + 1) * 128],
                            sb_wo[:, ci * 768 + nn * 384:
                                     ci * 768 + nn * 384 + 384],
                            start=(ci == 0), stop=(ci == 2))
                    nc.gpsimd.tensor_copy(ob[:, nn * 384:(nn + 1) * 384],
                                          pot[:])
                nc.gpsimd.dma_start(bounce_in[tb * 128:(tb + 1) * 128, :],
                                    ob[:])

            stg_tiles = {}

            def attn_part(qt, kb_lo, kb_hi, slot, pa, pb, last):
                """One kb-window of attention for all 6 heads of q-block qt.

                pa/pb: psum tiles holding the 4 ctx accumulation chains
                (qj 0,1 in pa; qj 2,3 in pb) at stream slot `slot`.
                """
                first = kb_lo == 0
                if not last and qt not in stg_tiles:
                    stg_tiles[qt] = stgp.tile([128, HPC * 260], F32, name="stg")
                stg = stg_tiles.get(qt)
                if last:
                    ctxts = [ctxtp.tile([128, 384], BF16, name="ctq")
                             for j in range(4)]
                for h in range(HPC):
                    pr, off = h // 2, (h % 2) * 64
                    for kb in range(kb_lo, kb_hi):
                        j = kb - 4 * qt
                        lo = max(j, 0) * 128
                        psc = pscore.tile([128, 512], F32)
                        nc.tensor.matmul(
                            psc[:, lo:],
                            sb_k[pr][off:off + 64, kb * 128:(kb + 1) * 128],
                            sb_q[pr][off:off + 64,
                                     qt * 512 + lo:(qt + 1) * 512],
                            start=True, stop=True)
                        et = etp.tile([128, 512], BF16)
                        with nc.allow_low_precision(reason="bf16 probs"):
                            nc.scalar.activation(et[:, lo:], psc[:, lo:], EXP)
                        if j >= 0:
                            with nc.allow_low_precision(reason="bf16 mask"):
                                nc.vector.tensor_mul(et[:, lo:lo + 128],
                                                     et[:, lo:lo + 128],
                                                     sb_mask[:])
                        for qj in range(4):
                            qc = 4 * qt + qj
                            if kb > qc:
                                continue
                            pt = pa if qj < 2 else pb
                            col = slot * 170 + (qj % 2) * 65
                            nc.tensor.matmul(
                                pt[:, col:col + 65],
                                et[:, qj * 128:(qj + 1) * 128],
                                sb_v[:, kb * VW + h * 65:kb * VW + h * 65 + 65],
                                start=(kb == kb_lo),
                                stop=(kb == qc or (kb == kb_hi - 1 and not last)),
                                skip_group_check=True)
                            if last and kb == qc:
                                rcp = rcpp.tile([128, 1], F32)
                                if stg is None:
                                    nc.vector.reciprocal(
                                        rcp[:], pt[:, col + 64:col + 65])
                                    with nc.allow_low_precision(reason="bf16 ctx"):
                                        nc.vector.tensor_scalar(
                                            ctxts[qj][:, h * 64:(h + 1) * 64],
                                            pt[:, col:col + 64],
                                            rcp[:], None, AluOpType.mult)
                                else:
                                    sc = scrp.tile([128, 65], F32)
                                    scol = h * 260 + (qj // 2) * 130 + (qj % 2) * 65
                                    nc.vector.tensor_add(
                                        sc[:], pt[:, col:col + 65],
                                        stg[:, scol:scol + 65])
                                    nc.vector.reciprocal(rcp[:], sc[:, 64:65])
                                    with nc.allow_low_precision(reason="bf16 ctx"):
                                        nc.vector.tensor_scalar(
                                            ctxts[qj][:, h * 64:(h + 1) * 64],
                                            sc[:, 0:64],
                                            rcp[:], None, AluOpType.mult)
                        yield
                    if not last:
                        # stage partial ctx sums to SBUF, free the psum slot
                        for pt, qp in ((pa, 0), (pb, 1)):
                            scol = h * 260 + qp * 130
                            sl = stg[:, scol:scol + 130]
                            ps = pt[:, slot * 170:slot * 170 + 130]
                            if first:
                                nc.vector.tensor_copy(sl, ps)
                            else:
                                nc.vector.tensor_add(sl, sl, ps)
                        yield
                if last:
                    for qj in range(4):
                        finish_tb(qt, qj, ctxts[qj])
                        yield
                    if not with_collective:
                        nc.sync.dma_start(
                            out[qt * 512:(qt + 1) * 512, :],
                            bounce_in[qt * 512:(qt + 1) * 512, :])

            def weave(gens):
                gens = list(gens)
                while gens:
                    alive = []
                    for g in gens:
                        try:
                            next(g)
                            alive.append(g)
                        except StopIteration:
                            continue

                    gens = alive

            # segment schedule: (tt streams) + (attention kb-windows whose
            # K/V blocks come from already-emitted tt streams)
            weave([proj_tt(0)])
            segs = [
                [(0, 0, 4, True), (1, 0, 4, False), (3, 0, 4, False)],
                [(1, 4, 8, True), (2, 0, 4, False), (3, 4, 8, False)],
                [(2, 4, 8, False), (3, 8, 12, False)],
                [(2, 8, 12, True), (3, 12, 16, True)],
            ]
            tts = {0: 1, 1: 2, 2: 3}
            for si, seg in enumerate(segs):
                pa = pctxp.tile([128, 512], F32, name="pa")
                pb = pctxp.tile([128, 512], F32, name="pb")
                gens = []
                if si in tts:
                    gens.append(proj_tt(tts[si]))
                for slot, (qt, klo, khi, last) in enumerate(seg):
                    gens.append(attn_part(qt, klo, khi, slot, pa, pb, last))
                weave(gens)

            if with_collective:
                nc.gpsimd.collective_compute(
                    "AllReduce", mybir.AluOpType.add,
                    replica_groups=[[0, 1], [2, 3], [4, 5], [6, 7]],
                    ins=[bounce_in.opt()], outs=[bounce_out.opt()])
                nc.sync.dma_start(out[:], bounce_out[:])
    nc.compile()
    return nc


def make_in_maps(x, w_q, w_k, w_v, w_o, token_positions):
    xn = np.asarray(x, np.float32)
    wqn = np.asarray(w_q, np.float32)
    wkn = np.asarray(w_k, np.float32)
    wvn = np.asarray(w_v, np.float32)
    won = np.asarray(w_o, np.float32)
    pos = np.asarray(token_positions).astype(np.float32)
    inv = THETA ** (-np.arange(32, dtype=np.float32) / 32.0)
    ang = inv[:, None] * pos[None, :]
    c32 = np.cos(ang).astype(np.float32)
    s32 = np.sin(ang).astype(np.float32)
    cosd = np.tile(c32, (4, 1)).astype(BF)
    sind = np.concatenate([s32, -s32, s32, -s32], axis=0).astype(BF)
    maskd = (np.arange(128)[:, None] <= np.arange(128)[None, :]).astype(BF)
    perm_eo = np.r_[0:64:2, 1:64:2]
    in_maps = []
    for c in range(N_CORES):
        b, hg = c // 2, c % 2
        heads = hg * HPC + np.arange(HPC)
        rows_eo = (heads[:, None] * 64 + perm_eo[None, :]).reshape(-1)
        # x: xd[p, tt*3072 + ck*512 + s] = x[b, tt*512+s, ck*128+p]
        xd_ = (xn[b].reshape(4, 512, 6, 128).transpose(3, 0, 2, 1)
               .reshape(128, 4 * 3072)).astype(BF)
        # wq/wk: w*d[p, ck*384 + j] = w_perm[j, ck*128+p]
        wql = wqn[rows_eo] * 0.125
        wqd_ = (wql.reshape(384, 6, 128).transpose(2, 1, 0)
                .reshape(128, 2304)).astype(BF)
        wkl = wkn[rows_eo]
        wkd_ = (wkl.reshape(384, 6, 128).transpose(2, 1, 0)
                .reshape(128, 2304)).astype(BF)
        # wv: wvd[p, ck*390 + h*65 + jj] = wv[(hg*6+h)*64 + jj, ck*128+p]
        wvl = np.zeros((VW, D), np.float32)
        for h in range(HPC):
            g = hg * HPC + h
            wvl[h * 65:h * 65 + 64] = wvn[g * 64:(g + 1) * 64]
        wvd_ = (wvl.reshape(VW, 6, 128).transpose(2, 1, 0)
                .reshape(128, 6 * VW)).astype(BF)
        # wo: wod[p, ci*768 + od] = w_o[od, hg*384 + ci*128 + p]
        wol = won[:, hg * 384:(hg + 1) * 384]
        wod_ = (wol.T.reshape(3, 128, 768).transpose(1, 0, 2)
                .reshape(128, 2304)).astype(BF)
        in_maps.append({
            "xd": xd_, "wqd": wqd_, "wkd": wkd_, "wvd": wvd_, "wod": wod_,
            "cos": cosd, "sin": sind, "mask": maskd,
        })
    return in_maps


def kernel(x, w_q, w_k, w_v, w_o, token_positions):
    global _NC
    if _NC is None:
        _NC = build_nc()
    in_maps = make_in_maps(x, w_q, w_k, w_v, w_o, token_positions)
    res = run_bass_kernel_spmd(_NC, in_maps, core_ids=list(range(N_CORES)))
    return np.stack([res.results[2 * b]["out"] for b in range(B)], axis=0)


# revision 6
# speedup vs baseline: 1.1556x; 1.0197x over previous
import numpy as np
import ml_dtypes
import concourse.bass as bass
import concourse.mybir as mybir
import concourse.tile as tile
from concourse import bacc
from concourse.bass_utils import run_bass_kernel_spmd
from concourse.alu_op_type import AluOpType

B, S, D = 4, 2048, 768
HPC = 6            # heads per core
PAIRS = 3
THETA = 10000.0
N_CORES = 8
F32 = mybir.dt.float32
BF16 = mybir.dt.bfloat16
BF = ml_dtypes.bfloat16
VW = HPC * 65      # 390: per-tb V tile width (6 heads x (64 dims + ones col))
EXP = mybir.ActivationFunctionType.Exp

_NC = None


def build_nc(with_collective=True):
    nc = bacc.Bacc("TRN2", target_bir_lowering=False, debug=False,
                   num_devices=N_CORES)
    xd = nc.dram_tensor("xd", [128, 4 * 3072], BF16, kind="ExternalInput")
    wqd = nc.dram_tensor("wqd", [128, 2304], BF16, kind="ExternalInput")
    wkd = nc.dram_tensor("wkd", [128, 2304], BF16, kind="ExternalInput")
    wvd = nc.dram_tensor("wvd", [128, 6 * VW], BF16, kind="ExternalInput")
    wod = nc.dram_tensor("wod", [128, 2304], BF16, kind="ExternalInput")
    cosd = nc.dram_tensor("cos", [128, S], BF16, kind="ExternalInput")
    sind = nc.dram_tensor("sin", [128, S], BF16, kind="ExternalInput")
    maskd = nc.dram_tensor("mask", [128, 128], BF16, kind="ExternalInput")
    out = nc.dram_tensor("out", [S, D], F32, kind="ExternalOutput")

    with tile.TileContext(nc) as tc:
        with tc.tile_pool(name="persist", bufs=1) as pp, \
             tc.tile_pool(name="dram", bufs=1, space="DRAM") as dpool, \
             tc.tile_pool(name="uhp", bufs=2) as uhp, \
             tc.tile_pool(name="swp", bufs=2) as swp, \
             tc.tile_pool(name="etp", bufs=4) as etp, \
             tc.tile_pool(name="ctxtp", bufs=8) as ctxtp, \
             tc.tile_pool(name="rcpp", bufs=4) as rcpp, \
             tc.tile_pool(name="scrp", bufs=4) as scrp, \
             tc.tile_pool(name="stgp", bufs=3) as stgp, \
             tc.tile_pool(name="obp", bufs=2) as obp, \
             tc.tile_pool(name="pgen", bufs=5, space="PSUM") as pgen, \
             tc.tile_pool(name="po", bufs=1, space="PSUM") as pop, \
             tc.tile_pool(name="pctx", bufs=1, space="PSUM") as pctxp:

            sb_q = [pp.tile([128, S], BF16, name=f"sb_q{i}") for i in range(PAIRS)]
            sb_k = [pp.tile([128, S], BF16, name=f"sb_k{i}") for i in range(PAIRS)]
            sb_v = pp.tile([128, 16 * VW], BF16)
            sb_ctx = [pp.tile([128, S], BF16, name=f"sb_ctx{i}") for i in range(PAIRS)]
            xcol = [pp.tile([128, 3072], BF16, name=f"xcol{i}") for i in range(4)]
            sb_wq = pp.tile([128, 2304], BF16)
            sb_wk = pp.tile([128, 2304], BF16)
            sb_wv = pp.tile([128, 6 * VW], BF16)
            sb_wo = pp.tile([128, 2304], BF16)
            sb_cos = pp.tile([128, S], BF16)
            sb_sin = pp.tile([128, S], BF16)
            sb_mask = pp.tile([128, 128], BF16)
            bounce_in = dpool.tile([S, D], F32)
            bounce_out = dpool.tile([S, D], F32)

            # input loads, most-urgent first
            nc.sync.dma_start(xcol[0][:], xd[:, 0:3072])
            nc.sync.dma_start(sb_wk[:], wkd[:])
            nc.sync.dma_start(sb_wq[:], wqd[:])
            nc.sync.dma_start(sb_cos[:], cosd[:])
            nc.sync.dma_start(sb_sin[:], sind[:])
            nc.sync.dma_start(sb_wv[:], wvd[:])
            nc.sync.dma_start(sb_mask[:], maskd[:])
            nc.sync.dma_start(xcol[1][:], xd[:, 3072:6144])
            nc.sync.dma_start(xcol[2][:], xd[:, 6144:9216])
            nc.sync.dma_start(xcol[3][:], xd[:, 9216:12288])
            nc.sync.dma_start(sb_wo[:], wod[:])

            def gen_tile():
                return pgen.tile([128, 512], F32, name="pgen_t")

            def proj_tt(tt):
                """QKV projection + RoPE for token block tt (512 tokens).

                K pairs first, swap+add per half so attention unblocks early;
                V chains interleaved to keep PE busy while DVE drains rope.
                """
                xc = xcol[tt]
                csl = sb_cos[:, tt * 512:(tt + 1) * 512]
                ssl = sb_sin[:, tt * 512:(tt + 1) * 512]
                uh = uhp.tile([128, 3072], BF16)
                swf = swp.tile([128, 3072], BF16)

                def qk_pair(wi, wsb, dst, pr):
                    pc = gen_tile()
                    for ck in range(6):
                        nc.tensor.matmul(
                            pc[:],
                            wsb[:, ck * 384 + pr * 128:
                                   ck * 384 + (pr + 1) * 128],
                            xc[:, ck * 512:(ck + 1) * 512],
                            start=(ck == 0), stop=(ck == 5))
                    with nc.allow_low_precision(reason="bf16 qk"):
                        nc.vector.tensor_mul(
                            dst[pr][:, tt * 512:(tt + 1) * 512], pc[:], csl)
                        nc.vector.tensor_mul(
                            uh[:, (wi * 3 + pr) * 512:(wi * 3 + pr + 1) * 512],
                            pc[:], ssl)

                def v_block(tj):
                    tb = tt * 4 + tj
                    pvt = gen_tile()
                    for ck in range(6):
                        nc.tensor.matmul(
                            pvt[:, 0:VW],
                            xc[:, ck * 512 + tj * 128:
                                  ck * 512 + tj * 128 + 128],
                            sb_wv[:, ck * VW:(ck + 1) * VW],
                            start=(ck == 0), stop=(ck == 5))
                    with nc.allow_low_precision(reason="bf16 v"):
                        nc.vector.tensor_copy(sb_v[:, tb * VW:(tb + 1) * VW],
                                              pvt[:, 0:VW])
                    nc.vector.memset(sb_v[:, tb * VW + 64:(tb + 1) * VW:65], 1.0)

                def swap_adds(wi, dst):
                    # partition swap of the sin part (32-part groups g <-> g^1)
                    for g in range(4):
                        nc.sync.dma_start(
                            swf[g * 32:(g + 1) * 32,
                                wi * 1536:(wi + 1) * 1536],
                            uh[(g ^ 1) * 32:((g ^ 1) + 1) * 32,
                               wi * 1536:(wi + 1) * 1536])
                    for pr in range(PAIRS):
                        sl = swf[:, (wi * 3 + pr) * 512:(wi * 3 + pr + 1) * 512]
                        d = dst[pr][:, tt * 512:(tt + 1) * 512]
                        with nc.allow_low_precision(reason="bf16 qk add"):
                            nc.vector.tensor_add(d, d, sl)

                for pr in range(PAIRS):
                    qk_pair(0, sb_wk, sb_k, pr)
                    yield
                    v_block(pr)
                    yield
                swap_adds(0, sb_k)
                yield
                for pr in range(PAIRS):
                    qk_pair(1, sb_wq, sb_q, pr)
                    yield
                    if pr == 0:
                        v_block(3)
                        yield
                swap_adds(1, sb_q)
                yield

            def finish_tb(qt, qj, ctxt):
                """Transpose ctx to [hd, tok], output projection, store."""
                tb = qt * 4 + qj
                for c in range(PAIRS):
                    nc.vector.transpose(
                        sb_ctx[c][:, tb * 128:(tb + 1) * 128],
                        ctxt[:, c * 128:(c + 1) * 128])
                ob = obp.tile([128, D], F32)
                for nn in range(2):
                    pot = pop.tile([128, 384], F32)
                    for ci in range(PAIRS):
                        nc.tensor.matmul(
                            pot[:],
                            sb_ctx[ci][:, tb * 128:(tb + 1) * 128],
                            sb_wo[:, ci * 768 + nn * 384:
                                     ci * 768 + nn * 384 + 384],
                            start=(ci == 0), stop=(ci == 2))
                    nc.vector.tensor_copy(ob[:, nn * 384:(nn + 1) * 384],
                                          pot[:])
                nc.gpsimd.dma_start(bounce_in[tb * 128:(tb + 1) * 128, :],
                                    ob[:])
                if not with_collective:
                    nc.sync.dma_start(out[tb * 128:(tb + 1) * 128, :],
                                      bounce_in[tb * 128:(tb + 1) * 128, :])

            stg_tiles = {}

            def attn_part(qt, kb_lo, kb_hi, slot, pa, pb, last):
                """One kb-window of attention for all 6 heads of q-block qt.

                pa/pb: psum tiles holding the 4 ctx accumulation chains
                (qj 0,1 in pa; qj 2,3 in pb) at stream slot `slot`.
                """
                first = kb_lo == 0
                if not last and qt not in stg_tiles:
                    stg_tiles[qt] = stgp.tile([128, HPC * 260], F32, name="stg")
                stg = stg_tiles.get(qt)
                if last:
                    ctxts = [ctxtp.tile([128, 384], BF16, name="ctq")
                             for j in range(4)]
                for h in range(HPC):
                    pr, off = h // 2, (h % 2) * 64
                    for kb in range(kb_lo, kb_hi):
                        j = kb - 4 * qt
                        lo = max(j, 0) * 128
                        psc = gen_tile()
                        nc.tensor.matmul(
                            psc[:, lo:],
                            sb_k[pr][off:off + 64, kb * 128:(kb + 1) * 128],
                            sb_q[pr][off:off + 64,
                                     qt * 512 + lo:(qt + 1) * 512],
                            start=True, stop=True)
                        et = etp.tile([128, 512], BF16)
                        with nc.allow_low_precision(reason="bf16 probs"):
                            nc.scalar.activation(et[:, lo:], psc[:, lo:], EXP)
                        if j >= 0:
                            with nc.allow_low_precision(reason="bf16 mask"):
                                nc.vector.tensor_mul(et[:, lo:lo + 128],
                                                     et[:, lo:lo + 128],
                                                     sb_mask[:])
                        for qj in range(4):
                            qc = 4 * qt + qj
                            if kb > qc:
                                continue
                            pt = pa if qj < 2 else pb
                            col = slot * 170 + (qj % 2) * 65
                            nc.tensor.matmul(
                                pt[:, col:col + 65],
                                et[:, qj * 128:(qj + 1) * 128],
                                sb_v[:, kb * VW + h * 65:kb * VW + h * 65 + 65],
                                start=(kb == kb_lo),
                                stop=(kb == qc or (kb == kb_hi - 1 and not last)),
                                skip_group_check=True)
                            if last and kb == qc:
                                rcp = rcpp.tile([128, 1], F32)
                                if stg is None:
                                    nc.vector.reciprocal(
                                        rcp[:], pt[:, col + 64:col + 65])
                                    with nc.allow_low_precision(reason="bf16 ctx"):
                                        nc.vector.tensor_scalar(
                                            ctxts[qj][:, h * 64:(h + 1) * 64],
                                            pt[:, col:col + 64],
                                            rcp[:], None, AluOpType.mult)
                                else:
                                    sc = scrp.tile([128, 65], F32)
                                    scol = h * 260 + (qj // 2) * 130 + (qj % 2) * 65
                                    nc.vector.tensor_add(
                                        sc[:], pt[:, col:col + 65],
                                        stg[:, scol:scol + 65])
                                    nc.vector.reciprocal(rcp[:], sc[:, 64:65])
                                    with nc.allow_low_precision(reason="bf16 ctx"):
                                        nc.vector.tensor_scalar(
                                            ctxts[qj][:, h * 64:(h + 1) * 64],
                                            sc[:, 0:64],
                                            rcp[:], None, AluOpType.mult)
                        yield
                    if not last:
                        # stage partial ctx sums to SBUF, free the psum slot
                        for pt, qp in ((pa, 0), (pb, 1)):
                            scol = h * 260 + qp * 130
                            sl = stg[:, scol:scol + 130]
                            ps = pt[:, slot * 170:slot * 170 + 130]
                            if first:
                                nc.vector.tensor_copy(sl, ps)
                            else:
                                nc.vector.tensor_add(sl, sl, ps)
                        yield
                if last:
                    for qj in range(4):
                        finish_tb(qt, qj, ctxts[qj])
                        yield

            def weave(gens):
                gens = list(gens)
                while gens:
                    alive = []
                    for g in gens:
                        try:
                            next(g)
                            alive.append(g)
                        except StopIteration:
                            continue

                    gens = alive

            # segment schedule: (tt streams) + (attention kb-windows whose
            # K/V blocks come from already-emitted tt streams)
            weave([proj_tt(0)])
            segs = [
                [(0, 0, 4, True), (1, 0, 4, False), (3, 0, 4, False)],
                [(1, 4, 8, True), (2, 0, 4, False), (3, 4, 8, False)],
                [(2, 4, 8, False), (3, 8, 12, False)],
                [(2, 8, 12, True), (3, 12, 16, True)],
            ]
            tts = {0: 1, 1: 2, 2: 3}
            for si, seg in enumerate(segs):
                pa = pctxp.tile([128, 512], F32, name="pa")
                pb = pctxp.tile([128, 512], F32, name="pb")
                gens = []
                if si in tts:
                    gens.append(proj_tt(tts[si]))
                for slot, (qt, klo, khi, last) in enumerate(seg):
                    gens.append(attn_part(qt, klo, khi, slot, pa, pb, last))
                weave(gens)

            if with_collective:
                nc.gpsimd.collective_compute(
                    "AllReduce", mybir.AluOpType.add,
                    replica_groups=[[0, 1], [2, 3], [4, 5], [6, 7]],
                    ins=[bounce_in.opt()], outs=[bounce_out.opt()])
                nc.sync.dma_start(out[:], bounce_out[:])
    nc.compile()
    return nc


def make_in_maps(x, w_q, w_k, w_v, w_o, token_positions):
    xn = np.asarray(x, np.float32)
    wqn = np.asarray(w_q, np.float32)
    wkn = np.asarray(w_k, np.float32)
    wvn = np.asarray(w_v, np.float32)
    won = np.asarray(w_o, np.float32)
    pos = np.asarray(token_positions).astype(np.float32)
    inv = THETA ** (-np.arange(32, dtype=np.float32) / 32.0)
    ang = inv[:, None] * pos[None, :]
    c32 = np.cos(ang).astype(np.float32)
    s32 = np.sin(ang).astype(np.float32)
    cosd = np.tile(c32, (4, 1)).astype(BF)
    sind = np.concatenate([s32, -s32, s32, -s32], axis=0).astype(BF)
    maskd = (np.arange(128)[:, None] <= np.arange(128)[None, :]).astype(BF)
    perm_eo = np.r_[0:64:2, 1:64:2]
    in_maps = []
    for c in range(N_CORES):
        b, hg = c // 2, c % 2
        heads = hg * HPC + np.arange(HPC)
        rows_eo = (heads[:, None] * 64 + perm_eo[None, :]).reshape(-1)
        # x: xd[p, tt*3072 + ck*512 + s] = x[b, tt*512+s, ck*128+p]
        xd_ = (xn[b].reshape(4, 512, 6, 128).transpose(3, 0, 2, 1)
               .reshape(128, 4 * 3072)).astype(BF)
        # wq/wk: w*d[p, ck*384 + j] = w_perm[j, ck*128+p]
        wql = wqn[rows_eo] * 0.125
        wqd_ = (wql.reshape(384, 6, 128).transpose(2, 1, 0)
                .reshape(128, 2304)).astype(BF)
        wkl = wkn[rows_eo]
        wkd_ = (wkl.reshape(384, 6, 128).transpose(2, 1, 0)
                .reshape(128, 2304)).astype(BF)
        # wv: wvd[p, ck*390 + h*65 + jj] = wv[(hg*6+h)*64 + jj, ck*128+p]
        wvl = np.zeros((VW, D), np.float32)
        for h in range(HPC):
            g = hg * HPC + h
            wvl[h * 65:h * 65 + 64] = wvn[g * 64:(g + 1) * 64]
        wvd_ = (wvl.reshape(VW, 6, 128).transpose(2, 1, 0)
                .reshape(128, 6 * VW)).astype(BF)
        # wo: wod[p, ci*768 + od] = w_o[od, hg*384 + ci*128 + p]
        wol = won[:, hg * 384:(hg + 1) * 384]
        wod_ = (wol.T.reshape(3, 128, 768).transpose(1, 0, 2)
                .reshape(128, 2304)).astype(BF)
        in_maps.append({
            "xd": xd_, "wqd": wqd_, "wkd": wkd_, "wvd": wvd_, "wod": wod_,
            "cos": cosd, "sin": sind, "mask": maskd,
        })
    return in_maps


def kernel(x, w_q, w_k, w_v, w_o, token_positions):
    global _NC
    if _NC is None:
        _NC = build_nc()
    in_maps = make_in_maps(x, w_q, w_k, w_v, w_o, token_positions)
    res = run_bass_kernel_spmd(_NC, in_maps, core_ids=list(range(N_CORES)))
    return np.stack([res.results[2 * b]["out"] for b in range(B)], axis=0)
